# revision 54
# baseline (speedup 1.0000x reference)
"""MultiHeadInfiniAttention Trainium2 kernel.

Reference computation (B=4, S=8192, D=1024, H=8, dk=dv=128, SEG=512):
  q,k,v = x @ w? + b?            -> per (seg, batch, head): [512, 128]
  scan over 16 segments with per-(b,h) memory state:
    sk = elu(k)+1; mem += sk^T v; z += SEG * sum_l(sk)
    sq = elu(q)+1; a_mem = (sq mem)/(sq z + 1e-5)
    attn = softmax(q k^T / sqrt(dk)); a_dot = attn v
    out = sig(beta) * mean_h(a_mem) + (1-sig(beta)) * mean_h(a_dot)

Sharding: 8 cores = 4 batches x 2 head-groups (4 heads each). Each core
computes a partial head-sum [S, 128]; host adds the two halves per batch.
The blend coefficients (from beta) are folded in on-chip: cb = sig/H scales
the memory matrix, 1/cd (cd = (1-sig)/H) is planted in the softmax
denominator column.

Layouts on chip (per segment t == seq tile of 512):
  xT tile   [128d, 512s] x 8 d-tiles  (f32 -> f32r via gpsimd copy)
  qT/kT     [128 dk, 512 l] f32r, transposed projection (w stationary);
            bias is a per-partition scalar in this orientation
  sqT/skT   [128 dk, 512 l] bf16   (elu+1 = relu(x) + min(exp(x), 1))
  sk_nat    [128 l, 128 dk] x4 per head: dma transpose of skT (dst offsets
            must be 64B-aligned -- that is why v_ext blocks stride by 160)
  v         natural-orientation projection (xT block stationary, N=512
            covers all 4 heads; bias folded in as a K=1 ones x bias-row
            matmul), scattered into v_ext blocks
  v_ext     persistent, 16 blocks x 160 cols: 0:128 = v, col 128 = 1/cd,
            col 129 = SEG (constant cols written once in the preamble)
  scores^T  psum [128 m, 512 l] = kT_block.T @ qT   (f32r, N=512 full rate)
  expT      [128 m, 512 l] bf16 = Exp(scores/sqrt(dk))
  a_dot     psum [128 l, 129] = sum_mt expT_mt.T @ v_ext_mt[:, :129]  (bf16)
  mem update psum [128 dk, 130] = sum_lt sk_nat.T @ v_ext_lt          (bf16)
  mem_f32   [128 dk, 130] per head, accumulated in SBUF f32
  mem_bf    [128 dk, 129] bf16: cols 0:128 = cb * mem, col 128 = z
  retrieval psum [128 l, 129] = sqT_block.T @ mem_bf                  (bf16)
  out_acc   [128 l, 128] x4 f32, accumulated over heads via
            out += (numer * recip(denom)) fused DVE ops; the reference's
            +1e-5 on the z denominator is dropped (denominator >= ~1e5)

Hardware notes learned the hard way:
  - matmul start=True clears the has_written state of the WHOLE psum bank,
    so a bank must host exactly one accumulation group at a time.
  - dma_start_transpose destinations must be 64B-aligned.
  - float32r operands must be produced by a compute op (DVE/ACT), not DMA.
"""

import sys

for _p in ("/opt/trn_rl_repo",):
    if _p not in sys.path:
        sys.path.insert(0, _p)

import numpy as np

import concourse.bass as bass
import concourse.tile as tile
import concourse.mybir as mybir
from concourse import bacc
from concourse.bass_utils import run_bass_kernel_spmd

F32 = mybir.dt.float32
F32R = mybir.dt.float32r
BF16 = mybir.dt.bfloat16
AF = mybir.ActivationFunctionType
ALU = mybir.AluOpType

B, S, D = 4, 8192, 1024
H_TOT, DK, DV, SEG = 8, 128, 128, 512
NSEG = S // SEG          # 16
HPC = 4                  # heads per core
NDT = D // 128           # 8 d-tiles
NCORES = 8
SCALE = 1.0 / float(np.sqrt(DK))

_CACHE = {}
DEBUG_TAPS = False
REPS = 1  # repeat the whole segment loop (for HW timing via deltas)


def _build():
    nc = bacc.Bacc("TRN2", target_bir_lowering=False, debug=False,
                   num_devices=NCORES)

    xT = nc.dram_tensor("xT", [D, S], F32, kind="ExternalInput")
    wq = nc.dram_tensor("wq", [D, HPC * DK], F32, kind="ExternalInput")
    wk = nc.dram_tensor("wk", [D, HPC * DK], F32, kind="ExternalInput")
    wv = nc.dram_tensor("wv", [D, HPC * DV], F32, kind="ExternalInput")
    bq = nc.dram_tensor("bq", [HPC, DK], F32, kind="ExternalInput")
    bk = nc.dram_tensor("bk", [HPC, DK], F32, kind="ExternalInput")
    bv = nc.dram_tensor("bv", [HPC, DV], F32, kind="ExternalInput")
    # coef columns: 0 = cb (= sig/H), 1 = 1/cd (cd = (1-sig)/H), broadcast 128x
    coef = nc.dram_tensor("coef", [128, 2], F32, kind="ExternalInput")
    out = nc.dram_tensor("out", [S, DV], F32, kind="ExternalOutput")

    dbg = {}
    if DEBUG_TAPS:
        for nm, shp in (("q", [128, 512]), ("k", [128, 512]),
                        ("sq", [128, 512]), ("skn", [128, 2048]),
                        ("vex", [128, 2048]), ("exp", [128, 2048]),
                        ("mem", [128, 129]), ("rtp", [128, 129]),
                        ("adp", [128, 129]), ("oacc", [128, 512]),
                        ("up", [128, 130])):
            dbg[nm] = nc.dram_tensor(f"dbg_{nm}", shp, BF16,
                                     kind="ExternalOutput")

    with tile.TileContext(nc) as tc:
        with tc.tile_pool(name="const", bufs=1) as cpool, \
             tc.tile_pool(name="work", bufs=2) as wpool, \
             tc.tile_pool(name="small", bufs=8) as spool, \
             tc.tile_pool(name="pp", bufs=3, space="PSUM") as proj_ps, \
             tc.tile_pool(name="sp", bufs=2, space="PSUM") as score_ps, \
             tc.tile_pool(name="sml_ps", bufs=3, space="PSUM") as sml_ps:
            upd_ps = ret_ps = adot_ps = sml_ps

            # ---- preamble: weights (f32r), biases, coef, mem state ----
            w_r = {}
            b_sb = {}
            for name, wd, bd in (("q", wq, bq), ("k", wk, bk), ("v", wv, bv)):
                wr = cpool.tile([128, NDT * 512], F32R, name=f"wr_{name}")
                for dt in range(NDT):
                    wst = wpool.tile([128, 512], F32, name=f"wst_{name}{dt}",
                                     tag="xtf", bufs=4)
                    nc.sync.dma_start(
                        wst[:], wd.ap()[dt * 128:(dt + 1) * 128, :])
                    nc.vector.tensor_copy(wr[:, dt * 512:(dt + 1) * 512],
                                          wst[:])
                w_r[name] = wr
                bt = cpool.tile([128, HPC], F32, name=f"b_{name}")
                for j in range(HPC):
                    nc.sync.dma_start(
                        bt[:, j:j + 1],
                        bd.ap()[j:j + 1, :].rearrange("a p -> p a"))
                b_sb[name] = bt

            coef_sb = cpool.tile([128, 2], F32, name="coef_sb")
            nc.sync.dma_start(coef_sb[:], coef.ap())

            # cz: col 0 = 1/cd (softmax denominator scale), col 1 = SEG
            # (z accumulation factor); constant over partitions (= l rows).
            cz = cpool.tile([128, 2], BF16, name="cz")
            nc.vector.tensor_copy(cz[:, 0:1], coef_sb[:, 1:2])
            nc.vector.memset(cz[:, 1:2], float(SEG))

            # K=1 operands folding the v bias into the natural-orientation
            # v projection: pp += ones.T @ bvrow
            ones_f = wpool.tile([1, 128], F32, name="ones_f", tag="ones_f",
                                bufs=1)
            nc.vector.memset(ones_f[:], 1.0)
            ones_r = cpool.tile([1, 128], F32R, name="ones_r")
            nc.vector.tensor_copy(ones_r[:], ones_f[:])
            bvst = wpool.tile([1, 512], F32, name="bvst", tag="bvst", bufs=1)
            for j in range(HPC):
                nc.sync.dma_start(bvst[0:1, j * 128:(j + 1) * 128],
                                  bv.ap()[j:j + 1, :])
            bvrow_r = cpool.tile([1, 512], F32R, name="bvrow_r")
            nc.vector.tensor_copy(bvrow_r[:], bvst[:])

            mem_f32 = cpool.tile([128, HPC * 130], F32, name="mem_f32")

            # two persistent v_ext buffers (alternate per segment); constant
            # cols written once here. Blocks of 160 cols: 0:128 = v,
            # col 128 = 1/cd, col 129 = SEG, 130:160 unused.
            v_ext_ab = []
            for i in range(2):
                ve = cpool.tile([128, HPC * 4 * 160], BF16, name=f"vext{i}")
                for blk in range(HPC * 4):
                    nc.gpsimd.tensor_copy(
                        ve[:, blk * 160 + 128:blk * 160 + 130], cz[:])
                v_ext_ab.append(ve)

            for rep in range(REPS):
              nc.vector.memset(mem_f32[:], 0.0)
              for t in range(NSEG):
                # ---- load x slice per d-tile, cast to f32r ----
                xt_r = wpool.tile([128, NDT * 512], F32R, name=f"xtr{t}",
                                  tag="xtr")
                for dt in range(NDT):
                    xt_f = wpool.tile([128, 512], F32, name=f"xtf{t}_{dt}",
                                      tag="xtf", bufs=4)
                    nc.sync.dma_start(
                        xt_f[:],
                        xT.ap()[dt * 128:(dt + 1) * 128,
                                t * 512:(t + 1) * 512])
                    nc.gpsimd.tensor_copy(xt_r[:, dt * 512:(dt + 1) * 512],
                                          xt_f[:])

                qT = wpool.tile([128, HPC * 512], F32R, name=f"qT{t}", tag="qT")
                kT = wpool.tile([128, HPC * 512], F32R, name=f"kT{t}", tag="kT")
                sqT = wpool.tile([128, HPC * 512], BF16, name=f"sqT{t}",
                                 tag="sqT")
                skT = wpool.tile([128, HPC * 512], BF16, name=f"skT{t}",
                                 tag="skT")
                sk_nat = wpool.tile([128, HPC * 512], BF16, name=f"skn{t}",
                                    tag="skn")
                v_ext = v_ext_ab[t % 2]

                # ---- q/k projections: psum[hd block, 512 seq], 8 d-tiles ----
                for name in ("q", "k"):
                    wr = w_r[name]
                    for hb in range(HPC):
                        pp = proj_ps.tile([128, 512], F32,
                                          name=f"pp{t}_{name}{hb}", tag="proj")
                        for dt in range(NDT):
                            nc.tensor.matmul(
                                pp[:],
                                wr[:, dt * 512 + hb * 128:dt * 512 + (hb + 1) * 128],
                                xt_r[:, dt * 512:(dt + 1) * 512],
                                start=(dt == 0), stop=(dt == NDT - 1))
                        bias = b_sb[name][:, hb:hb + 1]
                        sl = slice(hb * 512, (hb + 1) * 512)
                        raw = qT if name == "q" else kT
                        s_out = sqT if name == "q" else skT
                        # raw projection with bias (f32r, scores operand)
                        nc.scalar.activation(raw[:, sl], pp[:], AF.Identity,
                                             bias=bias)
                        # elu(x)+1 = relu(x) + min(exp(x), 1)
                        e_t = spool.tile([128, 512], BF16,
                                         name=f"e{t}_{name}{hb}", tag="e",
                                         bufs=3)
                        nc.scalar.activation(e_t[:], pp[:], AF.Exp,
                                             bias=bias)
                        r_t = spool.tile([128, 512], BF16,
                                         name=f"r{t}_{name}{hb}", tag="r",
                                         bufs=3)
                        nc.vector.tensor_scalar(r_t[:], pp[:], bias, 0.0,
                                                op0=ALU.add, op1=ALU.max)
                        nc.vector.scalar_tensor_tensor(
                            s_out[:, sl], e_t[:], 1.0, r_t[:],
                            op0=ALU.min, op1=ALU.add)
                        if name == "k":
                            for lt in range(4):
                                nc.sync.dma_start_transpose(
                                    sk_nat[:, (hb * 4 + lt) * 128:
                                           (hb * 4 + lt + 1) * 128],
                                    skT[:, hb * 512 + lt * 128:
                                        hb * 512 + (lt + 1) * 128])

                # ---- v projection, natural orientation: psum[l block,
                # 512 = 4 heads x 128], bias via K=1 ones matmul ----
                wr = w_r["v"]
                for lt in range(4):
                    pp = proj_ps.tile([128, 512], F32,
                                      name=f"pp{t}_v{lt}", tag="proj")
                    for dt in range(NDT):
                        nc.tensor.matmul(
                            pp[:],
                            xt_r[:, dt * 512 + lt * 128:dt * 512 + (lt + 1) * 128],
                            wr[:, dt * 512:(dt + 1) * 512],
                            start=(dt == 0), stop=False)
                    nc.tensor.matmul(pp[:], ones_r[:], bvrow_r[:],
                                     start=False, stop=True)
                    # scatter the 4 head blocks into v_ext (block id lt*4+h)
                    nc.scalar.activation(
                        v_ext[:, lt * 640:(lt + 1) * 640]
                        .rearrange("p (h c) -> p h c", c=160)[:, :, 0:128],
                        pp[:].rearrange("p (h c) -> p h c", c=128),
                        AF.Copy)


                if DEBUG_TAPS and t == 0:
                    dq = spool.tile([128, 512], BF16, name="dq", tag="dbg",
                                    bufs=2)
                    nc.vector.tensor_copy(dq[:], qT[:, 0:512])
                    nc.sync.dma_start(dbg["q"].ap(), dq[:])
                    dk_ = spool.tile([128, 512], BF16, name="dk_", tag="dbg",
                                     bufs=2)
                    nc.vector.tensor_copy(dk_[:], kT[:, 0:512])
                    nc.sync.dma_start(dbg["k"].ap(), dk_[:])
                    nc.sync.dma_start(dbg["sq"].ap(), sqT[:, 0:512])
                    nc.sync.dma_start(dbg["skn"].ap(), sk_nat[:])
                    nc.sync.dma_start(
                        dbg["vex"].ap().rearrange("p (b c) -> p b c", c=128),
                        v_ext[:].rearrange("p (lt h c) -> p (lt h) c",
                                           h=4, c=160)[:, :, 0:128])

                # ---- attention per head ----
                out_acc = wpool.tile([128, 4 * 128], F32, name=f"oac{t}",
                                     tag="oacc")
                nc.gpsimd.memset(out_acc[:], 0.0)
                for h in range(HPC):
                    hsl = slice(h * 512, (h + 1) * 512)
                    # scores^T -> exp
                    exp_sb = wpool.tile([128, 4 * 512], BF16,
                                        name=f"ex{t}_{h}", tag="exp")
                    for mb in range(4):
                        sps = score_ps.tile([128, 512], F32,
                                            name=f"sc{t}_{h}{mb}", tag="score")
                        nc.tensor.matmul(
                            sps[:],
                            kT[:, h * 512 + mb * 128:h * 512 + (mb + 1) * 128],
                            qT[:, hsl], start=True, stop=True)
                        nc.scalar.activation(exp_sb[:, mb * 512:(mb + 1) * 512],
                                             sps[:], AF.Exp, scale=SCALE)
                    # memory update (before retrieval)
                    up = upd_ps.tile([128, 130], F32, name=f"up{t}_{h}",
                                     tag="sml")
                    for lt in range(4):
                        blk = slice((h * 4 + lt) * 128, (h * 4 + lt + 1) * 128)
                        vbase = (lt * 4 + h) * 160
                        nc.tensor.matmul(
                            up[:], sk_nat[:, blk],
                            v_ext[:, vbase:vbase + 130],
                            start=(lt == 0), stop=(lt == 3))
                    if DEBUG_TAPS and t == 0 and h == 0:
                        dup = spool.tile([128, 130], BF16, name="dup",
                                         tag="dbg2", bufs=2)
                        nc.vector.tensor_copy(dup[:], up[:])
                        nc.sync.dma_start(dbg["up"].ap(), dup[:])
                    msl = slice(h * 130, h * 130 + 130)
                    nc.vector.tensor_add(mem_f32[:, msl], mem_f32[:, msl],
                                         up[:])
                    mem_bf = spool.tile([128, 129], BF16, name=f"mb{t}_{h}",
                                        tag="membf", bufs=2)
                    # cols 0:128 = cb * mem ; col 128 = z (unscaled)
                    nc.scalar.activation(mem_bf[:, 0:128],
                                         mem_f32[:, h * 130:h * 130 + 128],
                                         AF.Copy, scale=coef_sb[:, 0:1])
                    nc.vector.tensor_copy(mem_bf[:, 128:129],
                                          mem_f32[:, h * 130 + 129:h * 130 + 130])
                    if DEBUG_TAPS and t == 0 and h == 0:
                        nc.sync.dma_start(dbg["exp"].ap(), exp_sb[:])
                        nc.sync.dma_start(dbg["mem"].ap(), mem_bf[:])
                    for lb in range(4):
                        lsl = slice(h * 512 + lb * 128, h * 512 + (lb + 1) * 128)
                        rps = ret_ps.tile([128, 129], F32, name=f"rt{t}_{h}{lb}",
                                          tag="sml")
                        nc.tensor.matmul(rps[:], sqT[:, lsl], mem_bf[:],
                                         start=True, stop=True)
                        adp = adot_ps.tile([128, 129], F32,
                                           name=f"ad{t}_{h}{lb}", tag="sml")
                        for mt in range(4):
                            esl = slice(mt * 512 + lb * 128,
                                        mt * 512 + (lb + 1) * 128)
                            vbase = (mt * 4 + h) * 160
                            nc.tensor.matmul(
                                adp[:], exp_sb[:, esl],
                                v_ext[:, vbase:vbase + 129],
                                start=(mt == 0), stop=(mt == 3))
                        if DEBUG_TAPS and t == 0 and h == 0 and lb == 0:
                            drt = spool.tile([128, 129], BF16, name="drt",
                                             tag="dbg2", bufs=2)
                            nc.vector.tensor_copy(drt[:], rps[:])
                            nc.sync.dma_start(dbg["rtp"].ap(), drt[:])
                            dad = spool.tile([128, 129], BF16, name="dad",
                                             tag="dbg2", bufs=2)
                            nc.vector.tensor_copy(dad[:], adp[:])
                            nc.sync.dma_start(dbg["adp"].ap(), dad[:])
                        # epilogue: out += cb*numer/denz + numer2/dend
                        # (reference's +1e-5 is negligible: denz >= ~1e5)
                        rz = spool.tile([128, 1], F32, name=f"rz{t}_{h}{lb}",
                                        tag="rz", bufs=8)
                        nc.vector.reciprocal(rz[:], rps[:, 128:129])
                        rd = spool.tile([128, 1], F32, name=f"rd{t}_{h}{lb}",
                                        tag="rd", bufs=8)
                        nc.vector.reciprocal(rd[:], adp[:, 128:129])
                        osl = out_acc[:, lb * 128:(lb + 1) * 128]
                        nc.vector.scalar_tensor_tensor(
                            osl, rps[:, 0:128], rz[:], osl,
                            op0=ALU.mult, op1=ALU.add)
                        nc.vector.scalar_tensor_tensor(
                            osl, adp[:, 0:128], rd[:], osl,
                            op0=ALU.mult, op1=ALU.add)

                if DEBUG_TAPS and t == 0:
                    doa = spool.tile([128, 512], BF16, name="doa", tag="dbg",
                                     bufs=2)
                    nc.vector.tensor_copy(doa[:], out_acc[:])
                    nc.sync.dma_start(dbg["oacc"].ap(), doa[:])
                nc.sync.dma_start(
                    out.ap()[t * 512:(t + 1) * 512, :]
                       .rearrange("(lt p) v -> p lt v", p=128),
                    out_acc[:].rearrange("p (lt v) -> p lt v", v=128))

    nc.compile()
    return nc


def _get_compiled():
    if "nc" not in _CACHE:
        _CACHE["nc"] = _build()
    return _CACHE["nc"]


def kernel(x, wq, bq, wk, bk, wv, bv, beta):
    nc = _get_compiled()

    bsig = float(1.0 / (1.0 + np.exp(-np.float64(beta[0]))))
    cb = bsig / H_TOT
    cd = (1.0 - bsig) / H_TOT
    coef = np.empty((128, 2), np.float32)
    coef[:, 0] = cb
    coef[:, 1] = (1.0 / cd) if cd != 0.0 else np.inf

    xT_by_b = [np.ascontiguousarray(x[b].T).astype(np.float32, copy=False)
               for b in range(B)]
    in_maps = []
    for c in range(NCORES):
        b, hg = c // 2, c % 2
        sl = slice(hg * HPC * DK, (hg + 1) * HPC * DK)
        in_maps.append({
            "xT": xT_by_b[b],
            "wq": np.ascontiguousarray(wq[:, sl]),
            "wk": np.ascontiguousarray(wk[:, sl]),
            "wv": np.ascontiguousarray(wv[:, sl]),
            "bq": np.ascontiguousarray(bq[sl]).reshape(HPC, DK),
            "bk": np.ascontiguousarray(bk[sl]).reshape(HPC, DK),
            "bv": np.ascontiguousarray(bv[sl]).reshape(HPC, DV),
            "coef": coef,
        })

    res = run_bass_kernel_spmd(nc, in_maps, core_ids=list(range(NCORES)))
    out = np.empty((B, S, DV), np.float32)
    for b in range(B):
        out[b] = res.results[2 * b]["out"] + res.results[2 * b + 1]["out"]
    return out


if __name__ == "__main__":
    rng = np.random.default_rng(0)
    x = rng.normal(size=(B, S, D)).astype(np.float32)
    sc = 1.0 / np.sqrt(D)
    wq_ = (rng.normal(size=(D, 1024)) * sc).astype(np.float32)
    wk_ = (rng.normal(size=(D, 1024)) * sc).astype(np.float32)
    wv_ = (rng.normal(size=(D, 1024)) * sc).astype(np.float32)
    bq_ = (rng.normal(size=(1024,)) * 0.01).astype(np.float32)
    bk_ = (rng.normal(size=(1024,)) * 0.01).astype(np.float32)
    bv_ = (rng.normal(size=(1024,)) * 0.01).astype(np.float32)
    beta_ = np.zeros((1,), np.float32)
    o = kernel(x, wq_, bq_, wk_, bk_, wv_, bv_, beta_)
    print("out", o.shape, o.dtype, float(np.abs(o).max()))
